# revision 21
# baseline (speedup 1.0000x reference)
"""GRU memory-updater (scatter_memory) Trainium2 kernel.

Problem (torch.nn.GRUCell semantics, gate order r,z,n):
    h = S[idx]; h_new = GRUCell(messages, h)
    out = ones_like(S); out[idx] = h_new

Structure (8 cores, data-parallel over destination rows):
  - Host buckets updates by owner core, sorts by destination row, ships
    messages and the selected S rows pre-transposed (feature-major f16),
    and sign-flips the z-gate weights so the device computes z' = 1-z.
  - Device math per 512-token chunk, balanced across PE/ACT/DVE:
        r  = sigma(ps_r + b_r)                      [ACT]
        z' = sigma(ps_z' + b_z')                    [ACT]  (weights negated)
        t  = (ps_nh + b_hhn) * r                    [DVE STT]
        ps_u = I^T @ t + W_ihn @ msgs               [PE accumulation]
        n  = tanh(ps_u + b_ihn)                     [ACT]
        d  = n - h                                  [DVE TT f16]
        e  = z' * d                                 [DVE TT f16]
        flip: ps_eT = e^T via 4 transpose-matmuls   [PE]
        evac: scat = (hTok - 1) + ps_eT             [DVE STT]
              (= h_new - 1, since h + z'(n-h) - 1 = h_new - 1; hTok is
              the host-shipped token-major copy of h)
  - Output is f16: the ones-fill DMAs are interleaved with the input
    groups on the sync HWDGE ring (the Pool and ACT queues stay free),
    then CCE scatter-add of (h_new - 1); the host upcasts on unshard.
    messages and W_ih ship as fp8-e4m3 (gi-gate error ~0.9% rel, well
    under the 2e-2 budget), halving the message-stream bytes.
  - Scatter Q7 cost: consecutive-destination runs are decomposed into
    {4,3,2,1}-row segments; each segment is ONE descriptor (elem_size =
    k*D, elem_step = D over an overlapping-row DRAM view).  Per-class
    segment counts are EQUALIZED across cores (largest-run-first capped
    decomposition) so cross-core padding shrinks: 25 chunks instead of
    27.  Preps run back-to-back from ~6us on an otherwise-empty Pool
    queue; the last 128 quads get their own prep+trigger so the final
    chunk's scatter fires the moment it is computed.
"""

import numpy as np

import concourse.bacc as bacc
import concourse.mybir as mybir
import concourse.tile as tile
from concourse import bass_utils
from concourse.bass import AP
from concourse.masks import make_identity

N_NODES = 200000
M_MSGS = 100000
D = 128
NCORES = 8
RPC = N_NODES // NCORES
CH = 512
SPILL = 260

F16 = mybir.dt.float16
F8 = mybir.dt.float8e4
F32 = mybir.dt.float32
I16 = mybir.dt.int16

Alu = mybir.AluOpType
Act = mybir.ActivationFunctionType

CLASSES = (1, 2, 3, 4)  # segment sizes, region order in scat
# equalized per-core segment caps (feasible for this input; falls back to
# natural per-core maxima if not)
EQ_CAPS = {1: 3584, 2: 1664, 3: 896, 4: 768}


def _round_up(x: int, m: int) -> int:
    return (x + m - 1) // m * m


def build_gru_scatter(nc, Mp: int, counts: dict, V: int, lgroups: list[int]):
    """counts[k] = padded segment count for class k (multiple of 128).
    Mp = total padded tokens (multiple of CH)."""
    nch = Mp // CH
    assert sum(k * counts[k] for k in CLASSES) == Mp

    msgsT_d = nc.dram_tensor("msgsT", [D, Mp], F8, kind="ExternalInput").ap()
    hT_d = nc.dram_tensor("hT", [D, Mp], F16, kind="ExternalInput").ap()
    hTok_d = nc.dram_tensor("hTok", [128, Mp], F16, kind="ExternalInput").ap()
    sidx_d = {
        k: nc.dram_tensor(f"sidx{k}", [128, counts[k] // 16], I16,
                          kind="ExternalInput").ap()
        for k in CLASSES
    }
    wih_d = nc.dram_tensor("wihT", [D, 3 * D], F8, kind="ExternalInput").ap()
    whh_d = nc.dram_tensor("whhT", [D, 3 * D], F16, kind="ExternalInput").ap()
    bias_d = nc.dram_tensor("biases", [D, 4], F32, kind="ExternalInput").ap()
    out_d = nc.dram_tensor("out", [V, D], F16, kind="ExternalOutput").ap()
    # overlapping row views: class-k descriptor i spans rows [i, i+k)
    out_ap = {
        1: out_d,
        2: AP(out_d.tensor, 0, [[D, V - 1], [1, 2 * D]]),
        3: AP(out_d.tensor, 0, [[D, V - 2], [1, 3 * D]]),
        4: AP(out_d.tensor, 0, [[D, V - 3], [1, 4 * D]]),
    }

    with tile.TileContext(nc) as tc:
        with (
            tc.tile_pool(name="big", bufs=1) as big,
            tc.tile_pool(name="work", bufs=3) as work,
            tc.tile_pool(name="psum", bufs=1, space="PSUM") as pp,
        ):
            msgsT = big.tile([D, Mp], F8)
            hT = big.tile([D, Mp], F16)
            hTok = big.tile([128, Mp], F16)
            scat = big.tile([128, Mp], F16)  # token-major h_new - 1 staging

            wih = big.tile([D, 3 * D], F8)
            nc.sync.dma_start(out=wih[:], in_=wih_d)
            whh = big.tile([D, 3 * D], F16)
            nc.sync.dma_start(out=whh[:], in_=whh_d)
            g0 = lgroups[0] * CH
            sidx = {}
            for k in CLASSES:
                sidx[k] = big.tile([128, counts[k] // 16], I16,
                                   name=f"sidx{k}")
            # sidx1 gates the first (largest) scatter prep on the Q7 --
            # load it before anything else so descriptor gen starts early
            nc.sync.dma_start(out=sidx[1][:], in_=sidx_d[1])
            biases = big.tile([D, 4], F32)
            # first input group right behind sidx1 so compute starts early
            nc.sync.dma_start(out=msgsT[:, :g0], in_=msgsT_d[:, :g0])
            nc.sync.dma_start(out=hT[:, :g0], in_=hT_d[:, :g0])
            nc.sync.dma_start(out=hTok[:, :g0], in_=hTok_d[:, :g0])
            nc.sync.dma_start(out=biases[:], in_=bias_d)
            for k in (2, 3, 4):
                nc.sync.dma_start(out=sidx[k][:], in_=sidx_d[k])
            ident = big.tile([128, 128], F16)
            make_identity(nc, ident[:])
            ones = big.tile([128, 4096], F16)
            nc.vector.memset(ones[:], 1.0)

            # warm the ACT function table before the first real activation
            acttmp = big.tile([128, 4], F32)
            nc.scalar.activation(acttmp[:], biases[:], Act.Sigmoid)

            # input groups with ones-fill DMAs interleaved: the fill
            # (6.5MB) rides the same idle-after-issues sync ring and
            # finishes alongside the inputs instead of after them, so the
            # first scatter trigger is not fill-gated
            out_ones_view = out_d.rearrange("(p a) d -> p (a d)", p=128)
            fill_off = 0
            tok0 = g0
            for g in lgroups[1:]:
                ntok = g * CH
                nc.sync.dma_start(
                    out=msgsT[:, tok0 : tok0 + ntok],
                    in_=msgsT_d[:, tok0 : tok0 + ntok],
                )
                nc.sync.dma_start(
                    out=hT[:, tok0 : tok0 + ntok],
                    in_=hT_d[:, tok0 : tok0 + ntok],
                )
                nc.sync.dma_start(
                    out=hTok[:, tok0 : tok0 + ntok],
                    in_=hTok_d[:, tok0 : tok0 + ntok],
                )
                tok0 += ntok
                if fill_off < V:
                    blk = min(4096, V - fill_off)
                    nc.sync.dma_start(
                        out=out_ones_view[:, fill_off : fill_off + blk],
                        in_=ones[:, :blk],
                    )
                    fill_off += blk
            while fill_off < V:
                blk = min(4096, V - fill_off)
                nc.sync.dma_start(
                    out=out_ones_view[:, fill_off : fill_off + blk],
                    in_=ones[:, :blk],
                )
                fill_off += blk

            # ---- software-pipelined GRU compute ----
            # stage skews keep every in-order queue fed with ready work:
            #   PE:  mm5(c), u-mms(c-1), flip(c-2)
            #   ACT: n(c-2), [evac(c-3)], r(c), z'(c)
            #   DVE: t(c-1), [evac(c-3)], d(c-2), e(c-2)
            st_rz = {}
            st_nh = {}
            st_r = {}
            st_z = {}
            st_t = {}
            st_u = {}
            st_n = {}
            st_e = {}
            st_oT = {}

            def sl(c):
                return slice(c * CH, (c + 1) * CH)

            def stage_mm(c):
                ps_r = pp.tile([128, CH], F32, tag="ps_r", bufs=2)
                nc.tensor.matmul(ps_r[:], wih[:, 0:128], msgsT[:, sl(c)],
                                 start=True, stop=False)
                nc.tensor.matmul(ps_r[:], whh[:, 0:128], hT[:, sl(c)],
                                 start=False, stop=True)
                ps_z = pp.tile([128, CH], F32, tag="ps_z", bufs=2)
                nc.tensor.matmul(ps_z[:], wih[:, 128:256], msgsT[:, sl(c)],
                                 start=True, stop=False)
                nc.tensor.matmul(ps_z[:], whh[:, 128:256], hT[:, sl(c)],
                                 start=False, stop=True)
                st_rz[c] = (ps_r, ps_z)

            def stage_mm_nh(c):
                ps_nh = pp.tile([128, CH], F32, tag="ps_nh", bufs=1)
                nc.tensor.matmul(ps_nh[:], whh[:, 256:384], hT[:, sl(c)],
                                 start=True, stop=True)
                st_nh[c] = ps_nh

            def stage_rz(c):
                ps_r, ps_z = st_rz.pop(c)
                r = work.tile([128, CH], F16, tag="r", bufs=2)
                nc.scalar.activation(r[:], ps_r[:], Act.Sigmoid,
                                     bias=biases[:, 0:1])
                z = work.tile([128, CH], F16, tag="z", bufs=3)
                nc.scalar.activation(z[:], ps_z[:], Act.Sigmoid,
                                     bias=biases[:, 1:2])
                st_r[c] = r
                st_z[c] = z

            def stage_t(c):
                ps_nh = st_nh.pop(c)
                r = st_r.pop(c)
                t = work.tile([128, CH], F16, tag="t", bufs=2)
                nc.vector.scalar_tensor_tensor(
                    out=t[:], in0=ps_nh[:], scalar=biases[:, 3:4], in1=r[:],
                    op0=Alu.add, op1=Alu.mult,
                )
                st_t[c] = t

            def stage_u(c):
                t = st_t.pop(c)
                ps_u = pp.tile([128, CH], F32, tag="ps_u", bufs=2)
                nc.tensor.matmul(ps_u[:], ident[:], t[:], start=True, stop=False)
                nc.tensor.matmul(ps_u[:], wih[:, 256:384], msgsT[:, sl(c)],
                                 start=False, stop=True)
                st_u[c] = ps_u

            def stage_n(c):
                ps_u = st_u.pop(c)
                n = work.tile([128, CH], F16, tag="n", bufs=2)
                nc.scalar.activation(n[:], ps_u[:], Act.Tanh,
                                     bias=biases[:, 2:3])
                st_n[c] = n

            def stage_tail(c):
                n = st_n.pop(c)
                z = st_z.pop(c)
                d = work.tile([128, CH], F16, tag="d", bufs=2)
                nc.vector.tensor_tensor(out=d[:], in0=n[:], in1=hT[:, sl(c)],
                                        op=Alu.subtract)
                e = work.tile([128, CH], F16, tag="e", bufs=2)
                nc.vector.tensor_tensor(out=e[:], in0=z[:], in1=d[:],
                                        op=Alu.mult)
                st_e[c] = e

            def stage_flip(c):
                e = st_e.pop(c)
                ps_eT = pp.tile([128, CH], F32, tag="ps_eT", bufs=1)
                for k in range(CH // 128):
                    blk = slice(k * 128, (k + 1) * 128)
                    nc.tensor.matmul(ps_eT[:, blk], e[:, blk], ident[:],
                                     start=True, stop=True)
                st_oT[c] = ps_eT

            def stage_evac(c):
                ps_eT = st_oT.pop(c)
                nc.vector.scalar_tensor_tensor(
                    out=scat[:, sl(c)], in0=hTok[:, sl(c)], scalar=-1.0,
                    in1=ps_eT[:], op0=Alu.add, op1=Alu.add,
                )

            for c in range(nch):
                stage_mm(c)
                if c >= 2:
                    stage_n(c - 2)
                if c >= 3:
                    stage_evac(c - 3)
                stage_mm_nh(c)
                stage_rz(c)
                if c >= 1:
                    stage_t(c - 1)
                    stage_u(c - 1)
                if c >= 2:
                    stage_tail(c - 2)
                    stage_flip(c - 2)
            stage_t(nch - 1)
            stage_u(nch - 1)
            for c in (nch - 2, nch - 1):
                stage_n(c)
                stage_evac(c - 1)
                stage_tail(c)
                stage_flip(c)
            stage_evac(nch - 1)

            # ---- scatters: preps generate back-to-back on the Q7 with no
            # data gating (reads deferred to trigger time); each trigger
            # fires once its scat regions + the ones-fill are ready. ----
            scat_sem = nc.alloc_semaphore("scat_dma_sem")
            n_preps = 0

            def prep(k, idx0, nidx, base, queue_num=0):
                nonlocal n_preps
                nc.gpsimd.dma_scatter_add(
                    out_ap=out_ap[k],
                    in_ap=scat[:, base : base + k * nidx].rearrange(
                        "p (o n) -> p o n", n=k * D
                    ),
                    idxs_ap=sidx[k][:, idx0 // 16 : (idx0 + nidx) // 16],
                    num_idxs=nidx,
                    num_idxs_reg=nidx,
                    elem_size=k * D,
                    elem_step=None if k == 1 else D,
                    single_packet=False,
                    prepare_only=True,
                    sem=scat_sem,
                    queue_num=queue_num,
                )
                n_preps += 1

            base = 0
            for k in (1, 2):
                prep(k, 0, counts[k], base)
                base += k * counts[k]
            nc.gpsimd.trigger_dma(count=None)
            prep(3, 0, counts[3], base)
            base += 3 * counts[3]
            nc.gpsimd.trigger_dma(count=None)
            # quads: main body, then the final 128 (the last scat chunk) as
            # their own prep so the tail scatter fires immediately
            q_main = counts[4] - 128
            prep(4, 0, q_main, base)
            nc.gpsimd.trigger_dma(count=None)
            prep(4, q_main, 128, base + 4 * q_main)
            nc.gpsimd.trigger_dma(count=None)

            nc.gpsimd.wait_ge(scat_sem, 16 * n_preps)


def _wrap16(idx: np.ndarray) -> np.ndarray:
    n = idx.shape[0]
    w = idx.reshape(n // 16, 16).T.astype(np.int16)
    return np.tile(w, (8, 1))


def _runs(lidx_s: np.ndarray):
    """Maximal runs of consecutive values: list of (start_pos, length)."""
    cnt = len(lidx_s)
    out = []
    i = 0
    while i < cnt:
        j = i
        while j + 1 < cnt and lidx_s[j + 1] == lidx_s[j] + 1:
            j += 1
        out.append((i, j - i + 1))
        i = j + 1
    return out

def _capped_decomp(runs, caps):
    """Decompose runs into {4,3,2,1} pieces under per-class caps,
    processing longest runs first.  Returns {k: positions} or None."""
    pos = {k: [] for k in CLASSES}
    left = dict(caps)
    for start, L in sorted(runs, key=lambda r: -r[1]):
        i = start
        while L > 0:
            for k in (4, 3, 2, 1):
                if k <= L and left[k] > 0:
                    pos[k].append(i)
                    left[k] -= 1
                    i += k
                    L -= k
                    break
            else:
                return None
    return {k: np.array(sorted(v), np.int64) for k, v in pos.items()}


def prepare_inputs(messages, S, W_ih, W_hh, b_ih, b_hh, idx):
    messages = np.asarray(messages, dtype=np.float32)
    S = np.asarray(S, dtype=np.float32)
    W_ih = np.asarray(W_ih, dtype=np.float32)
    W_hh = np.asarray(W_hh, dtype=np.float32)
    b_ih = np.asarray(b_ih, dtype=np.float32)
    b_hh = np.asarray(b_hh, dtype=np.float32)
    idx = np.asarray(idx).astype(np.int64)

    owner = idx // RPC
    core_sel = []
    core_runs = []
    for c in range(NCORES):
        sel = np.nonzero(owner == c)[0]
        lidx = idx[sel] - c * RPC
        order = np.argsort(lidx, kind="stable")
        lidx_s = lidx[order]
        core_sel.append((sel[order], lidx_s))
        core_runs.append(_runs(lidx_s))

    # equalized caps if feasible for every core, else natural maxima
    decomps = [_capped_decomp(r, EQ_CAPS) for r in core_runs]
    if all(d is not None for d in decomps):
        counts = dict(EQ_CAPS)
    else:
        big = {k: 10**9 for k in CLASSES}
        decomps = [_capped_decomp(r, big) for r in core_runs]
        counts = {
            k: _round_up(max(max(len(d[k]) for d in decomps), 128), 128)
            for k in CLASSES
        }
    counts[4] = max(counts[4], 256)  # the last-128 split needs a main body
    # pad total tokens to a CH multiple via extra singles
    tok = sum(k * counts[k] for k in CLASSES)
    counts[1] += (-tok) % CH
    Mp = sum(k * counts[k] for k in CLASSES)
    V = _round_up(RPC + SPILL, 128)

    nch = Mp // CH
    lg = [1, 1, 2, 2, 3]
    rest = nch - sum(lg)
    ngr = 5
    bs = rest // ngr
    lgroups = lg + [bs + (1 if i < rest % ngr else 0) for i in range(ngr)]

    # z-gate sign flip: device computes z' = 1 - z
    Wf = np.concatenate([W_ih[0:128], -W_ih[128:256], W_ih[256:384]])
    Wh = np.concatenate([W_hh[0:128], -W_hh[128:256], W_hh[256:384]])
    f8 = mybir.dt.np(F8)
    wihT = np.ascontiguousarray(Wf.T).astype(f8)
    whhT = np.ascontiguousarray(Wh.astype(np.float16).T)
    biases = np.stack(
        [
            b_ih[0:128] + b_hh[0:128],
            -(b_ih[128:256] + b_hh[128:256]),
            b_ih[256:384],
            b_hh[256:384],
        ],
        axis=1,
    ).astype(np.float32)

    region_base = {}
    base = 0
    for k in CLASSES:
        region_base[k] = base
        base += k * counts[k]

    in_maps = []
    for c in range(NCORES):
        gsel, lidx_s = core_sel[c]
        pos = decomps[c]

        # token slot -> position in lidx_s (-1 = padding)
        slot_src = np.full(Mp, -1, np.int64)
        dsts = {}
        for k in CLASSES:
            p = pos[k]
            nk = len(p)
            j = np.arange(nk)
            o, q = j // 128, j % 128
            for m in range(k):
                slot_src[region_base[k] + o * (128 * k) + m * 128 + q] = p + m
            dst = np.empty(counts[k], np.int64)
            dst[:nk] = lidx_s[p]
            npad = counts[k] - nk
            if npad:
                dst[nk:] = RPC + k * (np.arange(npad) % ((SPILL - 8) // k))
            dsts[k] = dst

        src = np.clip(slot_src, 0, None)
        hsel = S[idx[gsel]][src].astype(np.float16)  # token-major [Mp, D]
        msgsT = np.ascontiguousarray(messages[gsel][src].T).astype(f8)
        hT = np.ascontiguousarray(hsel.T)
        # token-major layout matching scat: hTok[p, b*128+f] = h[b*128+p, f]
        hTok = np.ascontiguousarray(
            hsel.reshape(Mp // 128, 128, D).transpose(1, 0, 2).reshape(128, Mp)
        )

        im = {"msgsT": msgsT, "hT": hT, "hTok": hTok, "wihT": wihT,
              "whhT": whhT, "biases": biases}
        for k in CLASSES:
            im[f"sidx{k}"] = _wrap16(dsts[k])
        in_maps.append(im)
    return in_maps, Mp, counts, V, lgroups


def kernel(messages, S, W_ih, W_hh, b_ih, b_hh, idx):
    in_maps, Mp, counts, V, lgroups = prepare_inputs(
        messages, S, W_ih, W_hh, b_ih, b_hh, idx
    )

    nc = bacc.Bacc(
        "TRN2",
        target_bir_lowering=False,
        debug=False,
        enable_asserts=False,
        num_devices=NCORES,
        dynamic_dma_scratch_size=65536,
    )
    build_gru_scatter(nc, Mp, counts, V, lgroups)
    nc.compile()

    res = bass_utils.run_bass_kernel_spmd(
        nc, in_maps, core_ids=list(range(NCORES))
    )
    if res.exec_time_ns is not None:
        print(f"HW exec time: {res.exec_time_ns} ns")

    out = np.empty((N_NODES, D), dtype=np.float32)
    for c in range(NCORES):
        out[c * RPC : (c + 1) * RPC] = res.results[c]["out"][:RPC].astype(
            np.float32
        )
    return out
